# revision 10
# baseline (speedup 1.0000x reference)
"""CenterOfMassLoss Trainium2 kernel (fp8 DoubleRow edition).

Layout / strategy
-----------------
Inputs: predicted, target [1, 31, 2048, 2048] f32.  9 regions = 3 row-bands
x 3 col-bands, each 400x400, bands start at {200, 1000, 1500}.  Per
(channel, region) the loss needs center-of-mass moments of x^3 for both
tensors, the region-sum of target (raw), and the global mean of target.

Everything ships as fp8e4m3 (1 B/elem) and every matmul runs in DoubleRow
perf mode (0.5 PE cycles per output column, both operands fp8, pair axis =
2 extra contraction elements per partition):

  * target full image: 16*x.  Host permutes rows so every 128-row tile has
    the SAME band structure (p<25: band0, 25..50: band1, 50..75: band2,
    75..128: non-band rows; 400=16*25 and 848=16*53 divide exactly), so one
    stationary serves all 16 tiles; cols de-interleave to [even 1024 |
    odd 1024] so the DoubleRow pair n = image cols (2n, 2n+1) and a [4,
    1024] psum holds per-column-PAIR sums {all, band0, band1, band2}.
    Region col windows start at even cols, so pair sums preserve them.
    The stream is stored [ch][dma u][p][4 tiles][2048] so each DMA reads
    8 KB contiguous per partition (fat descriptors).
  * pred/target regions: host pre-cubes to z = 64*x^3 (fp8 error on z is
    1x instead of 3x), packs 3x3 regions to 1200 rows x 1200 cols (row
    g = 128t+p), de-interleaves each packed row to [even 600 | pad 8 |
    odd 600] (odd half at 16B-aligned pair stride -- ISA dual-fp8 rule).
    Stationary per row-tile: per row-band b the rows {S=1, A=(h>>4)-12,
    R=(h&15)-7.5, O=odd-member-only}; h-199.5 = 16*A + R exactly in
    e4m3.  The stationary is the same for all 3 col-bands, so ONE
    [12, 600] psum per tensor accumulates all 10 tiles via 2 bank-aligned
    matmuls each ([12,512] + [12,88]) -- 40 matmuls/channel for moments.
    Host recovers Sx = 16*A + R and Sy = sum((2n-199.5)*S[n]) + sum(O[n])
    per col-band (cols 200j..200j+200).
  * psum: rawsums [4, 1024] + pred [12, 600] + targ [12, 600] = 6 banks.
  * DoubleRow ISA rules honored: psum dst starts at partition 0, pair
    strides are even and 16B-aligned.

Per-core DMA is 28.3 MB (4 channels) round-robined over the sync /
scalar / gpsimd queues with fat (>= 6 KB) per-partition descriptors; PE
busy ~50 us (72 matmuls+ldweights per channel).  Channels across 8 cores
(7x4 + [28,29,30,dup]).  Final ~1k-flop combination on host in float64.
"""

import numpy as np
import ml_dtypes

E4 = ml_dtypes.float8_e4m3  # matches mybir.dt.float8e4

# ---------------- problem constants (hardcoded) ----------------
N_CORES = 8
CHANNELS = 31
H = W = 2048
NCH = 4  # channel slots per core
BS = [200, 1000, 1500]  # band starts (rows and cols)
RS = 400  # region side
NT_T = 16  # target row tiles of 128
NPB = 25  # band rows per target tile (3 bands -> partitions 0..75)
NNB = 53  # non-band rows per target tile (partitions 75..128)
NT_P = 10  # packed region row tiles (9 x 128 + 48 + pad)
PRED_N = 3 * RS  # 1200 packed cols (pre de-interleave)
CW = 1216  # de-interleaved packed width: [600 even | 8 pad | 600 odd]
ODD_OFF = 608  # odd-half offset (16B-aligned pair stride)
FUNDAMENTAL_INDEX = 4
FUNDA_WEIGHT = 5.0
TS = 16.0  # target full-image scale (dodges fp8 denormals)
CS = 64.0  # cube scale

# channel assignment per core: 7 cores x 4 channels + core 7 [28,29,30,30(dup)]
ASSIGN = [list(range(4 * k, 4 * k + 4)) for k in range(7)] + [[28, 29, 30, 30]]
VALID_SLOTS = [4, 4, 4, 4, 4, 4, 4, 3]  # dup slot ignored on host

# target row permutation: tile t partition p -> image row
_NONBAND = [r for r in range(H)
            if not any(s <= r < s + RS for s in BS)]  # 848 rows
assert len(_NONBAND) == NT_T * NNB


def _row_of(t, p):
    if p < 3 * NPB:
        b, q = p // NPB, p % NPB
        return BS[b] + NPB * t + q
    return _NONBAND[NNB * t + (p - 3 * NPB)]


def make_weights():
    """Stationary e4m3 matrices, pair-interleaved i-major with stride 16.

    wraw [128, 32]: single block for ALL target tiles (uniform row
        permutation): m=0 ones, m=1..3 band masks (p//25); both members.
    wcom [128, NT_P*32]: packed tile t block: for row g=128t+p<1200 with
        b=g//400, h=g%400: m=4b+0: 1, 4b+1: (h>>4)-12, 4b+2: (h&15)-7.5
        (both members), m=4b+3: 1 on odd member only.
    All values exactly representable in e4m3.
    """
    wraw = np.zeros((128, 32), dtype=np.float32)
    for p in range(128):
        for i in (0, 1):
            wraw[p, 16 * i + 0] = 1.0
            if p < 3 * NPB:
                wraw[p, 16 * i + 1 + p // NPB] = 1.0
    wcom = np.zeros((128, NT_P * 32), dtype=np.float32)
    for t in range(NT_P):
        for p in range(128):
            g = 128 * t + p
            if g < PRED_N:
                b, h = g // RS, g % RS
                for i in (0, 1):
                    wcom[p, 32 * t + 16 * i + 4 * b + 0] = 1.0
                    wcom[p, 32 * t + 16 * i + 4 * b + 1] = (h >> 4) - 12
                    wcom[p, 32 * t + 16 * i + 4 * b + 2] = (h & 15) - 7.5
                wcom[p, 32 * t + 16 * 1 + 4 * b + 3] = 1.0
    w8r = wraw.astype(E4)
    w8c = wcom.astype(E4)
    assert np.array_equal(w8r.astype(np.float32), wraw)
    assert np.array_equal(w8c.astype(np.float32), wcom)
    return w8r, w8c


def build_nc():
    """Build the per-core Bass program (same program on all 8 cores)."""
    import concourse.bacc as bacc
    import concourse.tile as tile
    from concourse import mybir

    F32 = mybir.dt.float32
    F8 = mybir.dt.float8e4
    DR = mybir.MatmulPerfMode.DoubleRow
    nc = bacc.Bacc("TRN2", debug=False)

    targ = nc.dram_tensor("targ", [NCH, 4, 128, 4, W], F8,
                          kind="ExternalInput")
    predc = nc.dram_tensor("predc", [NCH, 128, NT_P, CW], F8,
                           kind="ExternalInput")
    targc = nc.dram_tensor("targc", [NCH, 128, NT_P, CW], F8,
                           kind="ExternalInput")
    wraw_d = nc.dram_tensor("wraw", [128, 32], F8, kind="ExternalInput")
    wcom_d = nc.dram_tensor("wcom", [128, NT_P * 32], F8, kind="ExternalInput")
    momp_out = nc.dram_tensor("momp", [NCH, 12, 600], F32,
                              kind="ExternalOutput")
    momt_out = nc.dram_tensor("momt", [NCH, 12, 600], F32,
                              kind="ExternalOutput")
    raw_out = nc.dram_tensor("rawsums", [NCH, 4, W // 2], F32,
                             kind="ExternalOutput")

    with tile.TileContext(nc) as tc:
        with (
            tc.tile_pool(name="consts", bufs=1) as consts,
            tc.tile_pool(name="tpool", bufs=4) as tpool,
            tc.tile_pool(name="ppool", bufs=3) as ppool,
            tc.tile_pool(name="qpool", bufs=3) as qpool,
            tc.tile_pool(name="outsb", bufs=4) as outsb,
            tc.tile_pool(name="psum", bufs=1, space="PSUM") as psum,
        ):
            wraw_sb = consts.tile([128, 2, 16], F8)
            nc.scalar.dma_start(
                out=wraw_sb[:],
                in_=wraw_d[:].rearrange("p (two m) -> p two m", two=2),
            )
            wcom_sb = consts.tile([128, NT_P, 2, 16], F8)
            nc.scalar.dma_start(
                out=wcom_sb[:],
                in_=wcom_d[:].rearrange("p (t two m) -> p t two m", two=2, m=16),
            )

            queues = [nc.sync, nc.scalar, nc.gpsimd]
            for ci in range(NCH):
                rr = ci  # rotate queue assignment per channel
                # alternate r_ps banks so channel ci+1's raw matmuls don't
                # wait for channel ci's psum evacuation (6+2 banks in use)
                r_ps = psum.tile([4, W // 2], F32, tag=f"r_ps{ci % 2}")
                mom_p = psum.tile([12, 600], F32, tag="mom_p", name="mom_p")
                mom_t = psum.tile([12, 600], F32, tag="mom_t", name="mom_t")

                # ---- DMAs: pred cubes, target (raw), targ cubes ----
                ptiles = []
                for u in range(2):
                    ctile = ppool.tile([128, 5, CW], F8, tag="pctile",
                                       name=f"pctile{u}")
                    queues[(rr + u) % 3].dma_start(
                        out=ctile[:], in_=predc[ci, :, 5 * u:5 * u + 5, :])
                    ptiles.append(ctile)
                ttiles = []
                for u in range(4):
                    ttile = tpool.tile([128, 4, W], F8, tag="ttile")
                    queues[(rr + u + 2) % 3].dma_start(
                        out=ttile[:], in_=targ[ci, u])
                    ttiles.append(ttile)
                qtiles = []
                for u in range(2):
                    ctile = qpool.tile([128, 5, CW], F8, tag="tctile",
                                       name=f"tctile{u}")
                    queues[(rr + u) % 3].dma_start(
                        out=ctile[:], in_=targc[ci, :, 5 * u:5 * u + 5, :])
                    qtiles.append(ctile)

                # ---- PE: cube moments (one [12,600] psum per tensor) ----
                def cube_mms(ctiles, m):
                    for u in range(2):
                        for i in range(5):
                            t = 5 * u + i
                            pairs = ctiles[u][:, i, :].rearrange(
                                "p (two x) -> p two x", two=2
                            )
                            for c0, c1 in ((0, 512), (512, 600)):
                                nc.tensor.matmul(
                                    m[:, c0:c1],
                                    wcom_sb[:, t, :, :12],
                                    pairs[:, :, c0:c1],
                                    start=(t == 0),
                                    stop=(t == NT_P - 1),
                                    perf_mode=DR,
                                )

                cube_mms(ptiles, mom_p)
                # raw pair sums: all 32 matmuls share one stationary
                for u in range(4):
                    for i in range(4):
                        t = 4 * u + i
                        pairs = ttiles[u][:, i, :].rearrange(
                            "p (two n) -> p two n", two=2
                        )
                        for c in range(2):
                            nc.tensor.matmul(
                                r_ps[:, 512 * c:512 * (c + 1)],
                                wraw_sb[:, :, :4],
                                pairs[:, :, 512 * c:512 * (c + 1)],
                                start=(t == 0),
                                stop=(t == NT_T - 1),
                                perf_mode=DR,
                            )
                cube_mms(qtiles, mom_t)

                # ---- evacuate PSUM -> SBUF -> DRAM ----
                for m, dst in ((mom_p, momp_out), (mom_t, momt_out)):
                    mo = outsb.tile([12, 600], F32, tag="mo")
                    nc.scalar.copy(mo[:], m[:])
                    nc.gpsimd.dma_start(out=dst[ci], in_=mo[:])
                rout = outsb.tile([4, W // 2], F32, tag="rout")
                nc.scalar.copy(rout[:], r_ps[:])
                nc.gpsimd.dma_start(out=raw_out[ci], in_=rout[:])

    nc.compile()
    return nc


_NC = None


def _get_nc():
    global _NC
    if _NC is None:
        _NC = build_nc()
    return _NC


_F16_TO_E4 = None


def _lut_e4():
    """uint16 (f16 bits) -> uint8 (e4m3 bits) lookup table."""
    global _F16_TO_E4
    if _F16_TO_E4 is None:
        all16 = np.arange(65536, dtype=np.uint16).view(np.float16)
        with np.errstate(invalid="ignore"):
            _F16_TO_E4 = all16.astype(np.float32).astype(E4).view(np.uint8)
    return _F16_TO_E4


def to_e4(a_f32):
    """float32 array -> e4m3 (as uint8 bits) via f16 + LUT (fast path)."""
    lut = _lut_e4()
    f16 = a_f32.astype(np.float16)
    return lut[f16.view(np.uint16)]


# row permutation table: [NT_T, 128] image rows
_PERM = np.array([[_row_of(t, p) for p in range(128)] for t in range(NT_T)])


def pack_targ(t3, chs):
    """[31,H,W] f32 -> [NCH, 4, 128, 4, W] e4m3 of 16*x, rows permuted
    (uniform band structure), cols de-interleaved, DMA-contiguous."""
    out = np.empty((NCH, 4, 128, 4, W), dtype=np.uint8)
    for s, ch in enumerate(chs):
        q = to_e4(TS * t3[ch])  # [H, W] uint8
        d = np.empty_like(q)
        d[:, :W // 2] = q[:, 0::2]
        d[:, W // 2:] = q[:, 1::2]
        # tile t partition p <- image row _PERM[t, p]
        tiles = d[_PERM]  # [NT_T, 128, W]
        out[s] = tiles.reshape(4, 4, 128, W).transpose(0, 2, 1, 3)
    return out.view(E4)


def pack_cube(x3, chs):
    """[31,H,W] f32 -> [NCH, 128, NT_P, CW] e4m3 of 64*x^3, packed regions
    row-swizzled (row g = 128t+p) with whole-row col de-interleave."""
    pc = np.zeros((NCH, 128, NT_P, CW), dtype=np.uint8)
    rows = np.empty((PRED_N, PRED_N), dtype=np.float32)
    for s, ch in enumerate(chs):
        for b in range(3):
            for j in range(3):
                blk = x3[ch, BS[b]:BS[b] + RS, BS[j]:BS[j] + RS]
                rows[RS * b:RS * (b + 1), RS * j:RS * (j + 1)] = blk
        cube = to_e4(CS * (rows * rows * rows))
        d = np.zeros((PRED_N, CW), dtype=np.uint8)
        d[:, :PRED_N // 2] = cube[:, 0::2]
        d[:, ODD_OFF:ODD_OFF + PRED_N // 2] = cube[:, 1::2]
        full = d[:128 * (NT_P - 1)].reshape(NT_P - 1, 128, CW)
        pc[s, :, :NT_P - 1, :] = full.transpose(1, 0, 2)
        rem = PRED_N - 128 * (NT_P - 1)  # 48
        pc[s, :rem, NT_P - 1, :] = d[128 * (NT_P - 1):]
    return pc.view(E4)


def make_in_maps(predicted, target):
    """Pack full inputs into per-core in_maps (per-element transforms only)."""
    predicted = np.asarray(predicted, dtype=np.float32)
    target = np.asarray(target, dtype=np.float32)
    p3 = predicted[0]  # [31, H, W]
    t3 = target[0]
    wraw, wcom = make_weights()
    in_maps = []
    for k in range(N_CORES):
        chs = ASSIGN[k]
        in_maps.append({
            "targ": pack_targ(t3, chs),
            "predc": pack_cube(p3, chs),
            "targc": pack_cube(t3, chs),
            "wraw": wraw,
            "wcom": wcom,
        })
    return in_maps


def combine(results):
    """Host-side final math (float64) from per-core outputs."""
    n200 = np.arange(200, dtype=np.float64)
    wy = 2 * n200 - 199.5
    norms = np.zeros((9, CHANNELS), dtype=np.float64)
    rraw = np.zeros((9, CHANNELS), dtype=np.float64)
    gsum = 0.0
    for k in range(N_CORES):
        momp = np.asarray(results[k]["momp"], dtype=np.float64)
        momt = np.asarray(results[k]["momt"], dtype=np.float64)
        raw = np.asarray(results[k]["rawsums"], dtype=np.float64)
        for s in range(VALID_SLOTS[k]):
            ch = ASSIGN[k][s]
            gsum += raw[s, 0, :].sum() / TS
            for b in range(3):
                rb = raw[s, 1 + b]
                for j in range(3):
                    reg = 3 * b + j
                    rraw[reg, ch] = rb[BS[j] // 2:BS[j] // 2 + 200].sum() / TS
                    cen = []
                    for m in (momp, momt):
                        cols = slice(200 * j, 200 * (j + 1))
                        Srow = m[s, 4 * b + 0, cols]
                        S = Srow.sum()
                        Sx = 16 * m[s, 4 * b + 1, cols].sum() + \
                            m[s, 4 * b + 2, cols].sum()
                        Sy = (wy * Srow).sum() + m[s, 4 * b + 3, cols].sum()
                        cen.append((Sx / S, Sy / S))
                    dx = cen[0][0] - cen[1][0]
                    dy = cen[0][1] - cen[1][1]
                    norms[reg, ch] = np.sqrt(dx * dx + dy * dy)
    mean_target = gsum / (CHANNELS * H * W)
    weighting = rraw / (RS * RS) / mean_target  # [9, 31]
    terms = (norms * weighting).sum(axis=1)  # [9]
    terms[FUNDAMENTAL_INDEX] *= FUNDA_WEIGHT
    total = terms.sum() / (CHANNELS * 9)
    return np.float32(total)


def kernel(predicted, target):
    from concourse.bass_utils import run_bass_kernel_spmd

    nc = _get_nc()
    in_maps = make_in_maps(predicted, target)
    res = run_bass_kernel_spmd(nc, in_maps, list(range(N_CORES)))
    return np.asarray(combine(res.results), dtype=np.float32)


# revision 15
# speedup vs baseline: 1.1090x; 1.1090x over previous
"""CenterOfMassLoss Trainium2 kernel (fp8 DoubleRow edition).

Layout / strategy
-----------------
Inputs: predicted, target [1, 31, 2048, 2048] f32.  9 regions = 3 row-bands
x 3 col-bands, each 400x400, bands start at {200, 1000, 1500}.  Per
(channel, region) the loss needs center-of-mass moments of x^3 for both
tensors, the region-sum of target (raw), and the global mean of target.

Everything ships as fp8e4m3 (1 B/elem) and every matmul runs in DoubleRow
perf mode (0.5 PE cycles per output column, both operands fp8, pair axis =
2 extra contraction elements per partition):

  * target full image: 16*x.  Host permutes rows so every 128-row tile has
    the SAME band structure (p<25: band0, 25..50: band1, 50..75: band2,
    75..128: non-band rows; 400=16*25 and 848=16*53 divide exactly), so one
    stationary serves all 16 tiles; cols de-interleave to [even 1024 |
    odd 1024] so the DoubleRow pair n = image cols (2n, 2n+1) and a [4,
    1024] psum holds per-column-PAIR sums {all, band0, band1, band2}.
    Region col windows start at even cols, so pair sums preserve them.
    The stream is stored [ch][dma u][p][4 tiles][2048] so each DMA reads
    8 KB contiguous per partition (fat descriptors).
  * pred/target regions: host pre-cubes to z = 64*x^3 (fp8 error on z is
    1x instead of 3x), packs 3x3 regions to 1200 rows x 1200 cols (row
    g = 128t+p), de-interleaves each packed row to [even 600 | pad 8 |
    odd 600] (odd half at 16B-aligned pair stride -- ISA dual-fp8 rule).
    Stationary per row-tile: per row-band b the rows {S=1, A=(h>>4)-12,
    R=(h&15)-7.5, O=odd-member-only}; h-199.5 = 16*A + R exactly in
    e4m3.  The stationary is the same for all 3 col-bands, so ONE
    [12, 600] psum per tensor accumulates all 10 tiles via 2 bank-aligned
    matmuls each ([12,512] + [12,88]) -- 40 matmuls/channel for moments.
    Host recovers Sx = 16*A + R and Sy = sum((2n-199.5)*S[n]) + sum(O[n])
    per col-band (cols 200j..200j+200).
  * psum: rawsums [4, 1024] + pred [12, 600] + targ [12, 600] = 6 banks.
  * DoubleRow ISA rules honored: psum dst starts at partition 0, pair
    strides are even and 16B-aligned.

Per-core DMA is 28.3 MB (4 channels) round-robined over the sync /
scalar / gpsimd queues with fat (>= 6 KB) per-partition descriptors; PE
busy ~50 us (72 matmuls+ldweights per channel).  Channels across 8 cores
(7x4 + [28,29,30,dup]).  Final ~1k-flop combination on host in float64.
"""

import numpy as np
import ml_dtypes

E4 = ml_dtypes.float8_e4m3  # matches mybir.dt.float8e4

# ---------------- problem constants (hardcoded) ----------------
N_CORES = 8
CHANNELS = 31
H = W = 2048
NCH = 4  # channel slots per core
BS = [200, 1000, 1500]  # band starts (rows and cols)
RS = 400  # region side
NT_T = 16  # target row tiles of 128
NPB = 25  # band rows per target tile (3 bands -> partitions 0..75)
NNB = 53  # non-band rows per target tile (partitions 75..128)
NT_P = 10  # packed region row tiles (9 x 128 + 48 + pad)
PRED_N = 3 * RS  # 1200 packed cols (pre de-interleave)
CW = 1216  # de-interleaved packed width: [600 even | 8 pad | 600 odd]
ODD_OFF = 608  # odd-half offset (16B-aligned pair stride)
FUNDAMENTAL_INDEX = 4
FUNDA_WEIGHT = 5.0
TS = 16.0  # target full-image scale (dodges fp8 denormals)
CS = 64.0  # cube scale

# channel assignment per core: 7 cores x 4 channels + core 7 [28,29,30,30(dup)]
ASSIGN = [list(range(4 * k, 4 * k + 4)) for k in range(7)] + [[28, 29, 30, 30]]
VALID_SLOTS = [4, 4, 4, 4, 4, 4, 4, 3]  # dup slot ignored on host

# target row permutation: tile t partition p -> image row
_NONBAND = [r for r in range(H)
            if not any(s <= r < s + RS for s in BS)]  # 848 rows
assert len(_NONBAND) == NT_T * NNB


def _row_of(t, p):
    if p < 3 * NPB:
        b, q = p // NPB, p % NPB
        return BS[b] + NPB * t + q
    return _NONBAND[NNB * t + (p - 3 * NPB)]


def make_weights():
    """Stationary e4m3 matrices, pair-interleaved i-major with stride 16.

    wraw [128, 32]: single block for ALL target tiles (uniform row
        permutation): m=0 ones, m=1..3 band masks (p//25); both members.
    wcom [128, NT_P*32]: packed tile t block: for row g=128t+p<1200 with
        b=g//400, h=g%400: m=4b+0: 1, 4b+1: (h>>4)-12, 4b+2: (h&15)-7.5
        (both members), m=4b+3: 1 on odd member only.
    All values exactly representable in e4m3.
    """
    wraw = np.zeros((128, 32), dtype=np.float32)
    for p in range(128):
        for i in (0, 1):
            wraw[p, 16 * i + 0] = 1.0
            if p < 3 * NPB:
                wraw[p, 16 * i + 1 + p // NPB] = 1.0
    wcom = np.zeros((128, NT_P * 32), dtype=np.float32)
    for t in range(NT_P):
        for p in range(128):
            g = 128 * t + p
            if g < PRED_N:
                b, h = g // RS, g % RS
                for i in (0, 1):
                    wcom[p, 32 * t + 16 * i + 4 * b + 0] = 1.0
                    wcom[p, 32 * t + 16 * i + 4 * b + 1] = (h >> 4) - 12
                    wcom[p, 32 * t + 16 * i + 4 * b + 2] = (h & 15) - 7.5
                wcom[p, 32 * t + 16 * 1 + 4 * b + 3] = 1.0
    w8r = wraw.astype(E4)
    w8c = wcom.astype(E4)
    assert np.array_equal(w8r.astype(np.float32), wraw)
    assert np.array_equal(w8c.astype(np.float32), wcom)
    return w8r, w8c


def build_nc():
    """Build the per-core Bass program (same program on all 8 cores)."""
    import concourse.bacc as bacc
    import concourse.tile as tile
    from concourse import mybir

    F32 = mybir.dt.float32
    F8 = mybir.dt.float8e4
    DR = mybir.MatmulPerfMode.DoubleRow
    nc = bacc.Bacc("TRN2", debug=False)

    targ = nc.dram_tensor("targ", [NCH, 4, 128, 4, W], F8,
                          kind="ExternalInput")
    predc = nc.dram_tensor("predc", [NCH, 128, NT_P, CW], F8,
                           kind="ExternalInput")
    targc = nc.dram_tensor("targc", [NCH, 128, NT_P, CW], F8,
                           kind="ExternalInput")
    wraw_d = nc.dram_tensor("wraw", [128, 32], F8, kind="ExternalInput")
    wcom_d = nc.dram_tensor("wcom", [128, NT_P * 32], F8, kind="ExternalInput")
    momp_out = nc.dram_tensor("momp", [12, NCH, 600], F32,
                              kind="ExternalOutput")
    momt_out = nc.dram_tensor("momt", [12, NCH, 600], F32,
                              kind="ExternalOutput")
    raw_out = nc.dram_tensor("rawsums", [4, NCH, W // 2], F32,
                             kind="ExternalOutput")

    with tile.TileContext(nc) as tc:
        with (
            tc.tile_pool(name="consts", bufs=1) as consts,
            tc.tile_pool(name="tpool", bufs=4) as tpool,
            tc.tile_pool(name="ppool", bufs=3) as ppool,
            tc.tile_pool(name="qpool", bufs=3) as qpool,
            tc.tile_pool(name="psum", bufs=1, space="PSUM") as psum,
        ):
            wraw_sb = consts.tile([128, 2, 16], F8)
            nc.scalar.dma_start(
                out=wraw_sb[:],
                in_=wraw_d[:].rearrange("p (two m) -> p two m", two=2),
            )
            wcom_sb = consts.tile([128, NT_P, 2, 16], F8)
            nc.scalar.dma_start(
                out=wcom_sb[:],
                in_=wcom_d[:].rearrange("p (t two m) -> p t two m", two=2, m=16),
            )
            # output staging (flushed by 3 DMAs at the end so no output
            # trigger ever head-of-line-blocks an input DMA trigger)
            stg_mp = consts.tile([12, NCH, 600], F32)
            stg_mt = consts.tile([12, NCH, 600], F32)
            stg_raw = consts.tile([4, NCH, W // 2], F32)

            queues = [nc.sync, nc.scalar, nc.gpsimd]
            for ci in range(NCH):
                rr = ci  # rotate queue assignment per channel
                # alternate r_ps banks so channel ci+1's raw matmuls don't
                # wait for channel ci's psum evacuation (6+2 banks in use)
                r_ps = psum.tile([4, W // 2], F32, tag=f"r_ps{ci % 2}")
                mom_p = psum.tile([12, 600], F32, tag="mom_p", name="mom_p")
                mom_t = psum.tile([12, 600], F32, tag="mom_t", name="mom_t")

                # ---- DMAs: pred cubes, target (raw), targ cubes ----
                ptiles = []
                for u in range(2):
                    ctile = ppool.tile([128, 5, CW], F8, tag="pctile",
                                       name=f"pctile{u}")
                    queues[(rr + u) % 3].dma_start(
                        out=ctile[:], in_=predc[ci, :, 5 * u:5 * u + 5, :])
                    ptiles.append(ctile)
                ttiles = []
                for u in range(4):
                    ttile = tpool.tile([128, 4, W], F8, tag="ttile")
                    queues[(rr + u + 2) % 3].dma_start(
                        out=ttile[:], in_=targ[ci, u])
                    ttiles.append(ttile)
                qtiles = []
                for u in range(2):
                    ctile = qpool.tile([128, 5, CW], F8, tag="tctile",
                                       name=f"tctile{u}")
                    queues[(rr + u) % 3].dma_start(
                        out=ctile[:], in_=targc[ci, :, 5 * u:5 * u + 5, :])
                    qtiles.append(ctile)

                # ---- PE: cube moments (one [12,600] psum per tensor) ----
                def cube_mms(ctiles, m):
                    for u in range(2):
                        for i in range(5):
                            t = 5 * u + i
                            pairs = ctiles[u][:, i, :].rearrange(
                                "p (two x) -> p two x", two=2
                            )
                            for c0, c1 in ((0, 512), (512, 600)):
                                nc.tensor.matmul(
                                    m[:, c0:c1],
                                    wcom_sb[:, t, :, :12],
                                    pairs[:, :, c0:c1],
                                    start=(t == 0),
                                    stop=(t == NT_P - 1),
                                    perf_mode=DR,
                                )

                cube_mms(ptiles, mom_p)
                # raw pair sums: all 32 matmuls share one stationary
                for u in range(4):
                    for i in range(4):
                        t = 4 * u + i
                        pairs = ttiles[u][:, i, :].rearrange(
                            "p (two n) -> p two n", two=2
                        )
                        for c in range(2):
                            nc.tensor.matmul(
                                r_ps[:, 512 * c:512 * (c + 1)],
                                wraw_sb[:, :, :4],
                                pairs[:, :, 512 * c:512 * (c + 1)],
                                start=(t == 0),
                                stop=(t == NT_T - 1),
                                perf_mode=DR,
                            )
                cube_mms(qtiles, mom_t)

                # ---- evacuate PSUM -> SBUF staging (vector engine) ----
                nc.vector.tensor_copy(stg_mp[:, ci], mom_p[:])
                nc.vector.tensor_copy(stg_mt[:, ci], mom_t[:])
                nc.vector.tensor_copy(stg_raw[:, ci], r_ps[:])

            nc.sync.dma_start(out=momp_out[:], in_=stg_mp[:])
            nc.sync.dma_start(out=momt_out[:], in_=stg_mt[:])
            nc.sync.dma_start(out=raw_out[:], in_=stg_raw[:])

    nc.compile()
    return nc


_NC = None


def _get_nc():
    global _NC
    if _NC is None:
        _NC = build_nc()
    return _NC


_F16_TO_E4 = None


def _lut_e4():
    """uint16 (f16 bits) -> uint8 (e4m3 bits) lookup table."""
    global _F16_TO_E4
    if _F16_TO_E4 is None:
        all16 = np.arange(65536, dtype=np.uint16).view(np.float16)
        with np.errstate(invalid="ignore"):
            _F16_TO_E4 = all16.astype(np.float32).astype(E4).view(np.uint8)
    return _F16_TO_E4


def to_e4(a_f32):
    """float32 array -> e4m3 (as uint8 bits) via f16 + LUT (fast path)."""
    lut = _lut_e4()
    f16 = a_f32.astype(np.float16)
    return lut[f16.view(np.uint16)]


# row permutation table: [NT_T, 128] image rows
_PERM = np.array([[_row_of(t, p) for p in range(128)] for t in range(NT_T)])


def pack_targ(t3, chs):
    """[31,H,W] f32 -> [NCH, 4, 128, 4, W] e4m3 of 16*x, rows permuted
    (uniform band structure), cols de-interleaved, DMA-contiguous."""
    out = np.empty((NCH, 4, 128, 4, W), dtype=np.uint8)
    for s, ch in enumerate(chs):
        q = to_e4(TS * t3[ch])  # [H, W] uint8
        d = np.empty_like(q)
        d[:, :W // 2] = q[:, 0::2]
        d[:, W // 2:] = q[:, 1::2]
        # tile t partition p <- image row _PERM[t, p]
        tiles = d[_PERM]  # [NT_T, 128, W]
        out[s] = tiles.reshape(4, 4, 128, W).transpose(0, 2, 1, 3)
    return out.view(E4)


def pack_cube(x3, chs):
    """[31,H,W] f32 -> [NCH, 128, NT_P, CW] e4m3 of 64*x^3, packed regions
    row-swizzled (row g = 128t+p) with whole-row col de-interleave."""
    pc = np.zeros((NCH, 128, NT_P, CW), dtype=np.uint8)
    rows = np.empty((PRED_N, PRED_N), dtype=np.float32)
    for s, ch in enumerate(chs):
        for b in range(3):
            for j in range(3):
                blk = x3[ch, BS[b]:BS[b] + RS, BS[j]:BS[j] + RS]
                rows[RS * b:RS * (b + 1), RS * j:RS * (j + 1)] = blk
        cube = to_e4(CS * (rows * rows * rows))
        d = np.zeros((PRED_N, CW), dtype=np.uint8)
        d[:, :PRED_N // 2] = cube[:, 0::2]
        d[:, ODD_OFF:ODD_OFF + PRED_N // 2] = cube[:, 1::2]
        full = d[:128 * (NT_P - 1)].reshape(NT_P - 1, 128, CW)
        pc[s, :, :NT_P - 1, :] = full.transpose(1, 0, 2)
        rem = PRED_N - 128 * (NT_P - 1)  # 48
        pc[s, :rem, NT_P - 1, :] = d[128 * (NT_P - 1):]
    return pc.view(E4)


def make_in_maps(predicted, target):
    """Pack full inputs into per-core in_maps (per-element transforms only)."""
    predicted = np.asarray(predicted, dtype=np.float32)
    target = np.asarray(target, dtype=np.float32)
    p3 = predicted[0]  # [31, H, W]
    t3 = target[0]
    wraw, wcom = make_weights()
    in_maps = []
    for k in range(N_CORES):
        chs = ASSIGN[k]
        in_maps.append({
            "targ": pack_targ(t3, chs),
            "predc": pack_cube(p3, chs),
            "targc": pack_cube(t3, chs),
            "wraw": wraw,
            "wcom": wcom,
        })
    return in_maps


def combine(results):
    """Host-side final math (float64) from per-core outputs."""
    n200 = np.arange(200, dtype=np.float64)
    wy = 2 * n200 - 199.5
    norms = np.zeros((9, CHANNELS), dtype=np.float64)
    rraw = np.zeros((9, CHANNELS), dtype=np.float64)
    gsum = 0.0
    for k in range(N_CORES):
        momp = np.asarray(results[k]["momp"], dtype=np.float64)
        momt = np.asarray(results[k]["momt"], dtype=np.float64)
        raw = np.asarray(results[k]["rawsums"], dtype=np.float64)
        for s in range(VALID_SLOTS[k]):
            ch = ASSIGN[k][s]
            gsum += raw[0, s, :].sum() / TS
            for b in range(3):
                rb = raw[1 + b, s]
                for j in range(3):
                    reg = 3 * b + j
                    rraw[reg, ch] = rb[BS[j] // 2:BS[j] // 2 + 200].sum() / TS
                    cen = []
                    for m in (momp, momt):
                        cols = slice(200 * j, 200 * (j + 1))
                        Srow = m[4 * b + 0, s, cols]
                        S = Srow.sum()
                        Sx = 16 * m[4 * b + 1, s, cols].sum() + \
                            m[4 * b + 2, s, cols].sum()
                        Sy = (wy * Srow).sum() + m[4 * b + 3, s, cols].sum()
                        cen.append((Sx / S, Sy / S))
                    dx = cen[0][0] - cen[1][0]
                    dy = cen[0][1] - cen[1][1]
                    norms[reg, ch] = np.sqrt(dx * dx + dy * dy)
    mean_target = gsum / (CHANNELS * H * W)
    weighting = rraw / (RS * RS) / mean_target  # [9, 31]
    terms = (norms * weighting).sum(axis=1)  # [9]
    terms[FUNDAMENTAL_INDEX] *= FUNDA_WEIGHT
    total = terms.sum() / (CHANNELS * 9)
    return np.float32(total)


def kernel(predicted, target):
    from concourse.bass_utils import run_bass_kernel_spmd

    nc = _get_nc()
    in_maps = make_in_maps(predicted, target)
    res = run_bass_kernel_spmd(nc, in_maps, list(range(N_CORES)))
    return np.asarray(combine(res.results), dtype=np.float32)
